# revision 32
# baseline (speedup 1.0000x reference)
"""Trainium2 Bass kernel for BERTSpanNER boundary scores.

out[b,i,j,l] = min(cum[j+1,l]-cum[i,l], -EPS, begin[i,l], end[j,l]) on the
upper triangle (j>=i), else -1e9, where cum/begin/end derive from
log_softmax(x @ W + b) per label's I,B,L,U tag group.

Sharding: 8 cores = 4 batches x 2 label-halves (8 labels each). All cores run
one identical SPMD graph; per-core work differs only through input data (the
batch slice of x, and a label-permuted copy of W's columns).

Device writes only the computed j >= i0 region of each 128-row tile in an
l-major (S, LC, S) bf16 layout; the host fills the full constant -1e9 lower
triangle, transposes to [i, j, l] and upcasts to f32.

Engine split (measured: DVE bf16 ops run 2x, fp32 tensor_scalar with AP
scalars only 1x, gpsimd elementwise ~15x off): ScalarE produces
tsub = A[j]-C[i] for every label (activation, per-partition bias), DVE does
min-G (bf16 tensor_scalar 2x) and min-E2 (bf16 tensor_tensor 2x). The
cumsum A runs on the PE in bf16; C[i] = A[i-1] by PE transpose. Broadcasts
go through DRAM with per-partition-contiguous destinations (16KB packets).
"""
import os
import sys

for _p in ("/opt/trn_rl_repo", "/root/.axon_site/_ro/trn_rl_repo"):
    if os.path.isdir(_p) and _p not in sys.path:
        sys.path.insert(0, _p)

import numpy as np
import ml_dtypes
import concourse.bacc as bacc
import concourse.mybir as mybir
from concourse.bass import _add_dep_helper
from concourse.tile import TileContext
from concourse.bass_utils import run_bass_kernel_spmd
from concourse.alu_op_type import AluOpType

F32 = mybir.dt.float32
BF16 = mybir.dt.bfloat16
AF = mybir.ActivationFunctionType

B, S, H, NL = 4, 1024, 400, 16
NT = 1 + 4 * NL          # 65
EPS = 1e-8
NEG = -1e9
P = 128
NST = S // P             # 8 seq tiles
LC = NL // 2             # 8 labels per core
NKT = 4                  # k-tiles of H+1=401 (3x128 + 17, padded)
KT = [128, 128, 128, 17]
NW = NT + 4 * LC         # 97
QC = 256                 # x is loaded in 4 s-quarters of 256 columns

OUT_DT = BF16
OUT_NP = np.dtype("uint16")

_CACHED_NC = None


def _build():
    nc = bacc.Bacc()
    xTb = nc.declare_dram_parameter("xTb", [P, NKT * S], BF16, isOutput=False)
    Wcat = nc.declare_dram_parameter("Wcat", [P, NKT * NW + 1536], BF16,
                                     isOutput=False)
    eye = nc.declare_dram_parameter("eye", [P, P], F32, isOutput=False)
    out = nc.declare_dram_parameter("out", [S, LC * S], OUT_DT, isOutput=True)

    a_row_d = nc.dram_tensor("a_row_d", [LC, 512], F32)
    e2_row_d = nc.dram_tensor("e2_row_d", [LC, 512], BF16)
    a_row2_d = nc.dram_tensor("a_row2_d", [LC, S], F32)
    e2_row2_d = nc.dram_tensor("e2_row2_d", [LC, S], BF16)

    with TileContext(nc) as tc:
        with tc.tile_pool(name="const", bufs=1) as cpool, \
             tc.tile_pool(name="work", bufs=1) as wpool, \
             tc.tile_pool(name="sm", bufs=4) as smpool, \
             tc.tile_pool(name="ts", bufs=6) as tpool, \
             tc.tile_pool(name="u", bufs=2) as upool, \
             tc.tile_pool(name="oc", bufs=2) as opool, \
             tc.tile_pool(name="ps_proj", bufs=4, space="PSUM") as psp, \
             tc.tile_pool(name="ps_a", bufs=2, space="PSUM") as psa, \
             tc.tile_pool(name="ps_t", bufs=1, space="PSUM") as pst:

            # ---------------- input loads ------------------------------------
            # x arrives as 4 s-quarters so the projection can start early;
            # weights first on sync so they beat quarter 0.
            xk = cpool.tile([P, NKT * S], BF16, tag="xk")
            wc = cpool.tile([P, NKT * NW + 1536], BF16, tag="wc")
            eye_sb = cpool.tile([P, P], F32, tag="eye")
            nc.sync.dma_start(out=wc[:], in_=Wcat[:])
            triw_sb = wc[:, NKT * NW:]
            x_eng = [nc.scalar, nc.sync, nc.gpsimd, nc.scalar]
            for q in range(4):
                x_eng[q].dma_start(out=xk[:, q * NKT * QC:(q + 1) * NKT * QC],
                                   in_=xTb[:, q * NKT * QC:(q + 1) * NKT * QC])
            nc.gpsimd.dma_start(out=eye_sb[:], in_=eye[:])

            # ---------------- projection matmuls + exp -----------------------
            E_all = wpool.tile([P, NST * NW], F32, tag="e_all")
            exp_list = []
            for t in range(NST):
                q, r = t // 2, t % 2
                ps = psp.tile([P, 512], F32, tag="ps_proj")
                for ki, kt in enumerate(KT):
                    col0 = q * NKT * QC + ki * QC + r * P
                    nc.tensor.matmul(ps[:, :NW], xk[0:kt, col0:col0 + P],
                                     wc[0:kt, ki * NW:(ki + 1) * NW],
                                     start=ki == 0, stop=ki == NKT - 1)
                # logits are tiny (|x@W| < ~4), exp needs no max-shift
                ei = nc.scalar.activation(E_all[:, t * NW:(t + 1) * NW],
                                          ps[:, :NW], AF.Exp)
                exp_list.append(ei)

            # ---------------- batched softmax reductions (DVE) ---------------
            ssum = smpool.tile([P, NST], F32, tag="ssum")
            for t in range(NST):
                nc.vector.tensor_reduce(ssum[:, t:t + 1],
                                        E_all[:, t * NW: t * NW + NT],
                                        mybir.AxisListType.X, AluOpType.add)

            E4 = E_all[:].rearrange("p (t w) -> p t w", w=NW)
            el = E4[:, :, NT:NW].rearrange("p t (l k) -> p t l k", k=4)
            t01 = smpool.tile([P, NST * LC], F32, tag="t01")
            t01v = t01[:].rearrange("p (t l) -> p t l", l=LC)
            nc.vector.tensor_tensor(t01v, el[:, :, :, 0], el[:, :, :, 1],
                                    AluOpType.add)
            t23 = smpool.tile([P, NST * LC], F32, tag="t23")
            t23v = t23[:].rearrange("p (t l) -> p t l", l=LC)
            nc.vector.tensor_tensor(t23v, el[:, :, :, 2], el[:, :, :, 3],
                                    AluOpType.add)
            sum4 = smpool.tile([P, NST * LC], F32, tag="sum4")
            nc.vector.tensor_tensor(sum4[:], t01[:], t23[:], AluOpType.add)
            begE = smpool.tile([P, NST * LC], F32, tag="begE")
            begEv = begE[:].rearrange("p (t l) -> p t l", l=LC)
            nc.vector.tensor_tensor(begEv, el[:, :, :, 1], el[:, :, :, 3],
                                    AluOpType.add)
            endE = smpool.tile([P, NST * LC], F32, tag="endE")
            endEv = endE[:].rearrange("p (t l) -> p t l", l=LC)
            nc.vector.tensor_tensor(endEv, el[:, :, :, 2], el[:, :, :, 3],
                                    AluOpType.add)

            # ---------------- all Lns (one act-table switch) -----------------
            lsum = smpool.tile([P, NST], F32, tag="lsum")
            lnS4 = smpool.tile([P, NST * LC], F32, tag="lnS4")
            lnB = smpool.tile([P, NST * LC], F32, tag="lnB")
            lnE = smpool.tile([P, NST * LC], F32, tag="lnE")
            lns = [nc.scalar.activation(lsum[:], ssum[:], AF.Ln),
                   nc.scalar.activation(lnE[:], endE[:], AF.Ln),
                   nc.scalar.activation(lnS4[:], sum4[:], AF.Ln),
                   nc.scalar.activation(lnB[:], begE[:], AF.Ln)]
            for _li in lns:
                _add_dep_helper(_li.ins, exp_list[-1].ins, True, "ln after exps")

            # ---------------- normalize: x - ln(sum) (DVE) -------------------
            ins_all = wpool.tile([P, NST * LC], BF16, tag="ins_all")
            G_all = wpool.tile([P, NST * LC], F32, tag="g_all")
            E2_all = wpool.tile([P, NST * LC], F32, tag="e2_all")
            for t in range(NST):
                csl = slice(t * LC, (t + 1) * LC)
                ls = lsum[:, t:t + 1]
                nc.vector.tensor_scalar(E2_all[:, csl], lnE[:, csl], ls, -EPS,
                                        AluOpType.subtract, AluOpType.min)
                nc.vector.tensor_scalar(ins_all[:, csl], lnS4[:, csl], ls, None,
                                        AluOpType.subtract)
                nc.vector.tensor_scalar(G_all[:, csl], lnB[:, csl], ls, None,
                                        AluOpType.subtract)

            # ---------------- E2 transpose to rows (PE + DVE copy) -----------
            E2_colT = wpool.tile([LC, S], BF16, tag="e2_colt")
            for t in range(NST):
                csl = slice(t * LC, (t + 1) * LC)
                tp = pst.tile([LC, P], F32, tag="ps_e2t")
                nc.tensor.transpose(tp[:], E2_all[:, csl], eye_sb[:])
                nc.vector.tensor_copy(E2_colT[:, t * P:(t + 1) * P], tp[:])

            # ---------------- A[l,j] = cumsum of inside, via PE --------------
            # A_colT[:, 0] = 0; A_colT[:, 1+j] = A[j]  (C[i]=A[i-1] at col i)
            A_colT = wpool.tile([LC, S + P], F32, tag="a_colt")
            nc.vector.memset(A_colT[:, 0:1], 0.0)
            a_copies = []
            for jc in (1, 0):           # high half first: unblocks tiles 7..4
                jc0 = jc * 512
                ap = psa.tile([P, 512], F32, tag="ps_a")
                tmax = (jc0 + 512) // P
                for ti in range(tmax):
                    o = ti * P - jc0
                    if o < 0:
                        rhs = triw_sb[:, 1024:1536]          # all ones
                    else:
                        rhs = triw_sb[:, 512 - o:1024 - o]   # k <= j' - o
                    nc.tensor.matmul(ap[:LC, :], ins_all[:, ti * LC:(ti + 1) * LC],
                                     rhs, start=ti == 0, stop=ti == tmax - 1)
                cp = nc.vector.tensor_copy(A_colT[:, 1 + jc0:1 + jc0 + 512],
                                           ap[:LC, :])
                a_copies.append(cp)

            # ---------------- C[i] = A[i-1] via PE transpose -----------------
            C_all = wpool.tile([P, NST * LC], F32, tag="c_all")
            ncs_all = wpool.tile([P, NST * LC], F32, tag="ncs_all")
            for t in range(NST - 1, -1, -1):
                csl = slice(t * LC, (t + 1) * LC)
                tp = pst.tile([P, LC], F32, tag="ps_ct")
                nc.tensor.transpose(tp[:], A_colT[:, t * P: (t + 1) * P],
                                    eye_sb[0:LC, 0:LC])
                nc.vector.tensor_copy(C_all[:, csl], tp[:])
                nc.vector.tensor_scalar(ncs_all[:, csl], C_all[:, csl], -1.0,
                                        None, AluOpType.mult)

            # ---------------- broadcasts (DRAM roundtrip) --------------------
            # All destinations are per-partition contiguous (16KB packets).
            A_hi = wpool.tile([P, LC * 512], F32, tag="a_hi")
            E2_hi = wpool.tile([P, LC * 512], BF16, tag="e2_hi")
            A_b = wpool.tile([P, LC * S], F32, tag="a_b")
            E2_b = wpool.tile([P, LC * S], BF16, tag="e2_b")
            A_hi3 = A_hi[:].rearrange("p (l j) -> p l j", l=LC)
            E2_hi3 = E2_hi[:].rearrange("p (l j) -> p l j", l=LC)
            E2_b3 = E2_b[:].rearrange("p (l j) -> p l j", l=LC)

            def _bcast(eng, dst, src_flat, deps, note):
                r = eng.dma_start(out=dst,
                                  in_=src_flat.partition_broadcast(P))
                for dp in deps:
                    _add_dep_helper(r.ins, dp.ins, True, note)
                return r

            # sync: half1 writes + hi-tile broadcast
            aw1 = nc.sync.dma_start(out=a_row_d[:, :], in_=A_colT[:, 513:1025])
            _add_dep_helper(aw1.ins, a_copies[0].ins, True, "a w1")
            aw1b = nc.sync.dma_start(out=a_row2_d[:, 512:1024],
                                     in_=A_colT[:, 513:1025])
            _add_dep_helper(aw1b.ins, a_copies[0].ins, True, "a w1b")
            _bcast(nc.sync, A_hi[:],
                   a_row_d[:, :].rearrange("l j -> (l j)"), [aw1], "a hi")
            # scalar: half0 write + low-label full broadcast
            aw0 = nc.scalar.dma_start(out=a_row2_d[:, 0:512],
                                      in_=A_colT[:, 1:513])
            _add_dep_helper(aw0.ins, a_copies[1].ins, True, "a w0")
            H4 = LC // 2
            _bcast(nc.scalar, A_b[:, :H4 * S],
                   a_row2_d[0:H4, :].rearrange("l j -> (l j)"),
                   [aw0, aw1b], "a full lo-labels")
            _bcast(nc.sync, A_b[:, H4 * S:],
                   a_row2_d[H4:LC, :].rearrange("l j -> (l j)"),
                   [aw0, aw1b], "a full hi-labels")
            # gpsimd: all E2 staging + broadcasts
            ew1 = nc.gpsimd.dma_start(out=e2_row_d[:, :],
                                      in_=E2_colT[:, 512:1024])
            ew1b = nc.gpsimd.dma_start(out=e2_row2_d[:, 512:1024],
                                       in_=E2_colT[:, 512:1024])
            ew0 = nc.gpsimd.dma_start(out=e2_row2_d[:, 0:512],
                                      in_=E2_colT[:, 0:512])
            _bcast(nc.gpsimd, E2_hi[:],
                   e2_row_d[:, :].rearrange("l j -> (l j)"), [ew1], "e2 hi")
            _bcast(nc.gpsimd, E2_b[:, :H4 * S],
                   e2_row2_d[0:H4, :].rearrange("l j -> (l j)"),
                   [ew0, ew1b], "e2 full lo-labels")
            _bcast(nc.gpsimd, E2_b[:, H4 * S:],
                   e2_row2_d[H4:LC, :].rearrange("l j -> (l j)"),
                   [ew0, ew1b], "e2 full hi-labels")

            # ---------------- main span sweep (l-major, bf16) ----------------
            out3 = out[:].rearrange("(t p) f -> t p f", p=P)
            out_eng = [nc.sync, nc.gpsimd]
            # hi tiles first (need only the hi broadcasts), t=0 next so its
            # big output DMA overlaps the remaining sweep, smallest tile last
            # to minimize the final-DMA tail
            for oi, t in enumerate((4, 5, 6, 0, 1, 2, 3, 7)):
                i0 = t * P
                W = S - i0
                if t >= 4:
                    a_of, a_sb, a_w = i0 - 512, A_hi, 512
                    e2_op = E2_hi3[:, :, i0 - 512:512]
                else:
                    a_of, a_sb, a_w = i0, A_b, S
                    e2_op = E2_b3[:, :, i0:S]
                oc = opool.tile([P, LC * W], OUT_DT, tag="oc")
                u = upool.tile([P, LC * W], OUT_DT, tag="u")
                # label 7 fused on DVE (fp32 1x, keeps ScalarE under its
                # saturation point); 0-6 via ScalarE tsub + DVE 2x min
                for l in (7,):
                    ci = t * LC + l
                    nc.vector.tensor_scalar(u[:, l * W:(l + 1) * W],
                                            a_sb[:, l * a_w + a_of:(l + 1) * a_w],
                                            C_all[:, ci:ci + 1],
                                            G_all[:, ci:ci + 1],
                                            AluOpType.subtract, AluOpType.min)
                for l in range(7):
                    ci = t * LC + l
                    tsub = tpool.tile([P, W], OUT_DT, tag="tsub")
                    nc.scalar.activation(tsub[:],
                                         a_sb[:, l * a_w + a_of:(l + 1) * a_w],
                                         AF.Identity, bias=ncs_all[:, ci:ci + 1])
                    nc.vector.tensor_scalar(u[:, l * W:(l + 1) * W], tsub[:],
                                            G_all[:, ci:ci + 1], None,
                                            AluOpType.min)
                u3 = u[:].rearrange("p (l j) -> p l j", j=W)
                oc3 = oc[:].rearrange("p (l j) -> p l j", j=W)
                nc.vector.tensor_tensor(oc3[:], u3[:], e2_op, AluOpType.min)
                dst = out3[t, :, :].rearrange("p (l j) -> p l j", l=LC)[:, :, i0:S]
                out_eng[oi % 2].dma_start(out=dst, in_=oc3)

    nc.compile()
    return nc


def _host_inputs(x, W, b):
    """Build per-core input maps. Core c: batch c//2, label half c%2."""
    x = np.asarray(x, dtype=np.float32)
    W = np.asarray(W, dtype=np.float32)
    b = np.asarray(b, dtype=np.float32)

    Wb = np.concatenate([W, b[None, :]], axis=0)          # (401, 65)
    eye = np.eye(P, dtype=np.float32)
    triw = np.zeros((P, 1536), np.float32)
    cc = np.arange(1536)[None, :]
    kk = np.arange(P)[:, None]
    triw[kk <= cc - 512] = 1.0
    triw = triw.astype(ml_dtypes.bfloat16)

    in_maps = []
    for c in range(8):
        bb, h = c // 2, c % 2
        cols = []
        for l in range(LC):
            base = 1 + 4 * (h * LC + l)
            cols.extend(range(base, base + 4))
        xTb = np.concatenate([x[bb].T, np.ones((1, S), np.float32)], axis=0)
        wcat = np.concatenate([Wb, Wb[:, cols]], axis=1)          # (401, 97)
        xp = np.zeros((NKT * P, S), np.float32)
        xp[:H + 1] = xTb
        # [P, (q, ki, 256)]: s-quarter major, so quarter DMAs are contiguous
        xp = (xp.reshape(NKT, P, 4, QC).transpose(1, 2, 0, 3)
                .reshape(P, NKT * S))
        wp = np.zeros((NKT * P, NW), np.float32)
        wp[:H + 1] = wcat
        wp = np.ascontiguousarray(
            wp.reshape(NKT, P, NW).transpose(1, 0, 2).reshape(P, NKT * NW))
        wp = np.concatenate([wp.astype(ml_dtypes.bfloat16), triw], axis=1)
        in_maps.append({
            "xTb": np.ascontiguousarray(xp).astype(ml_dtypes.bfloat16),
            "Wcat": wp,
            "eye": eye,
        })
    return in_maps


def _from_out_dt(a):
    if OUT_DT == F32:
        return a
    return (a.astype(np.uint32) << 16).view(np.float32)


def kernel(x, mask, W, b, _collect=None):
    global _CACHED_NC
    if _CACHED_NC is None:
        _CACHED_NC = _build()
    nc = _CACHED_NC
    in_maps = _host_inputs(x, W, b)
    res = run_bass_kernel_spmd(nc, in_maps, list(range(8)))
    if _collect is not None:
        _collect.append(res)
    outf = np.empty((B, S, S, NL), dtype=np.float32)
    for c in range(8):
        bb, h = c // 2, c % 2
        o = res.results[c]["out"]
        if o.dtype != np.float32:
            o = _from_out_dt(o.view(OUT_NP) if o.dtype != OUT_NP else o)
        o = o.reshape(S, LC, S)                       # [i, l, j]
        outf[bb, :, :, h * LC:(h + 1) * LC] = o.transpose(0, 2, 1)
    # full constant lower triangle filled on host (device only computes
    # j >= i0 per row tile and leaves j < i garbage within the tile)
    for i in range(1, S):
        outf[:, i, :i, :] = NEG
    return outf


# revision 34
# speedup vs baseline: 1.0996x; 1.0996x over previous
"""Trainium2 Bass kernel for BERTSpanNER boundary scores.

out[b,i,j,l] = min(cum[j+1,l]-cum[i,l], -EPS, begin[i,l], end[j,l]) on the
upper triangle (j>=i), else -1e9, where cum/begin/end derive from
log_softmax(x @ W + b) per label's I,B,L,U tag group.

Sharding: 8 cores = 4 batches x 2 label-halves (8 labels each). All cores run
one identical SPMD graph; per-core work differs only through input data (the
batch slice of x, and a label-permuted copy of W's columns).

Device writes only the computed j >= i0 region of each 128-row tile in an
l-major (S, LC, S) bf16 layout; the host fills the full constant -1e9 lower
triangle, transposes to [i, j, l] and upcasts to f32.

Engine split (measured: DVE bf16 ops run 2x, fp32 tensor_scalar with AP
scalars only 1x, gpsimd elementwise ~15x off): ScalarE produces
tsub = A[j]-C[i] for every label (activation, per-partition bias), DVE does
min-G (bf16 tensor_scalar 2x) and min-E2 (bf16 tensor_tensor 2x). The
cumsum A runs on the PE in bf16; C[i] = A[i-1] by PE transpose. Broadcasts
go through DRAM with per-partition-contiguous destinations (16KB packets).
"""
import os
import sys

for _p in ("/opt/trn_rl_repo", "/root/.axon_site/_ro/trn_rl_repo"):
    if os.path.isdir(_p) and _p not in sys.path:
        sys.path.insert(0, _p)

import numpy as np
import ml_dtypes
import concourse.bacc as bacc
import concourse.mybir as mybir
from concourse.bass import _add_dep_helper
from concourse.tile import TileContext
from concourse.bass_utils import run_bass_kernel_spmd
from concourse.alu_op_type import AluOpType

F32 = mybir.dt.float32
BF16 = mybir.dt.bfloat16
AF = mybir.ActivationFunctionType

B, S, H, NL = 4, 1024, 400, 16
NT = 1 + 4 * NL          # 65
EPS = 1e-8
NEG = -1e9
P = 128
NST = S // P             # 8 seq tiles
LC = NL // 2             # 8 labels per core
NKT = 4                  # k-tiles of H+1=401 (3x128 + 17, padded)
KT = [128, 128, 128, 17]
NW = NT + 4 * LC         # 97
QC = 256                 # x is loaded in 4 s-quarters of 256 columns

OUT_DT = BF16
OUT_NP = np.dtype("uint16")

_CACHED_NC = None


def _build():
    nc = bacc.Bacc()
    xTb = nc.declare_dram_parameter("xTb", [P, NKT * S], BF16, isOutput=False)
    Wcat = nc.declare_dram_parameter("Wcat", [P, NKT * NW + 1536], BF16,
                                     isOutput=False)
    eye = nc.declare_dram_parameter("eye", [P, P], F32, isOutput=False)
    out = nc.declare_dram_parameter("out", [S, LC * S], OUT_DT, isOutput=True)

    a_row_d = nc.dram_tensor("a_row_d", [LC, 512], F32)
    e2_row_d = nc.dram_tensor("e2_row_d", [LC, 512], BF16)
    a_row2_d = nc.dram_tensor("a_row2_d", [LC, S], F32)
    e2_row2_d = nc.dram_tensor("e2_row2_d", [LC, S], BF16)

    with TileContext(nc) as tc:
        with tc.tile_pool(name="const", bufs=1) as cpool, \
             tc.tile_pool(name="work", bufs=1) as wpool, \
             tc.tile_pool(name="sm", bufs=4) as smpool, \
             tc.tile_pool(name="ts", bufs=6) as tpool, \
             tc.tile_pool(name="u", bufs=2) as upool, \
             tc.tile_pool(name="oc", bufs=2) as opool, \
             tc.tile_pool(name="ps_proj", bufs=4, space="PSUM") as psp, \
             tc.tile_pool(name="ps_a", bufs=2, space="PSUM") as psa, \
             tc.tile_pool(name="ps_t", bufs=1, space="PSUM") as pst:

            # ---------------- input loads ------------------------------------
            # x arrives as 4 s-quarters so the projection can start early;
            # weights first on sync so they beat quarter 0.
            xk = cpool.tile([P, NKT * S], BF16, tag="xk")
            wc = cpool.tile([P, NKT * NW + 1536], BF16, tag="wc")
            eye_sb = cpool.tile([P, P], F32, tag="eye")
            nc.sync.dma_start(out=wc[:], in_=Wcat[:])
            triw_sb = wc[:, NKT * NW:]
            x_eng = [nc.scalar, nc.sync, nc.gpsimd, nc.scalar]
            for q in range(4):
                x_eng[q].dma_start(out=xk[:, q * NKT * QC:(q + 1) * NKT * QC],
                                   in_=xTb[:, q * NKT * QC:(q + 1) * NKT * QC])
            nc.gpsimd.dma_start(out=eye_sb[:], in_=eye[:])

            # ---------------- projection matmuls + exp -----------------------
            E_all = wpool.tile([P, NST * NW], F32, tag="e_all")
            exp_list = []
            for t in range(NST):
                q, r = t // 2, t % 2
                ps = psp.tile([P, 512], F32, tag="ps_proj")
                for ki, kt in enumerate(KT):
                    col0 = q * NKT * QC + ki * QC + r * P
                    nc.tensor.matmul(ps[:, :NW], xk[0:kt, col0:col0 + P],
                                     wc[0:kt, ki * NW:(ki + 1) * NW],
                                     start=ki == 0, stop=ki == NKT - 1)
                # logits are tiny (|x@W| < ~4), exp needs no max-shift
                ei = nc.scalar.activation(E_all[:, t * NW:(t + 1) * NW],
                                          ps[:, :NW], AF.Exp)
                exp_list.append(ei)

            # ---------------- batched softmax reductions (DVE) ---------------
            ssum = smpool.tile([P, NST], F32, tag="ssum")
            for t in range(NST):
                nc.vector.tensor_reduce(ssum[:, t:t + 1],
                                        E_all[:, t * NW: t * NW + NT],
                                        mybir.AxisListType.X, AluOpType.add)

            E4 = E_all[:].rearrange("p (t w) -> p t w", w=NW)
            el = E4[:, :, NT:NW].rearrange("p t (l k) -> p t l k", k=4)
            t01 = smpool.tile([P, NST * LC], F32, tag="t01")
            t01v = t01[:].rearrange("p (t l) -> p t l", l=LC)
            nc.vector.tensor_tensor(t01v, el[:, :, :, 0], el[:, :, :, 1],
                                    AluOpType.add)
            t23 = smpool.tile([P, NST * LC], F32, tag="t23")
            t23v = t23[:].rearrange("p (t l) -> p t l", l=LC)
            nc.vector.tensor_tensor(t23v, el[:, :, :, 2], el[:, :, :, 3],
                                    AluOpType.add)
            sum4 = smpool.tile([P, NST * LC], F32, tag="sum4")
            nc.vector.tensor_tensor(sum4[:], t01[:], t23[:], AluOpType.add)
            begE = smpool.tile([P, NST * LC], F32, tag="begE")
            begEv = begE[:].rearrange("p (t l) -> p t l", l=LC)
            nc.vector.tensor_tensor(begEv, el[:, :, :, 1], el[:, :, :, 3],
                                    AluOpType.add)
            endE = smpool.tile([P, NST * LC], F32, tag="endE")
            endEv = endE[:].rearrange("p (t l) -> p t l", l=LC)
            nc.vector.tensor_tensor(endEv, el[:, :, :, 2], el[:, :, :, 3],
                                    AluOpType.add)

            # ---------------- all Lns (one act-table switch) -----------------
            lsum = smpool.tile([P, NST], F32, tag="lsum")
            lnS4 = smpool.tile([P, NST * LC], F32, tag="lnS4")
            lnB = smpool.tile([P, NST * LC], F32, tag="lnB")
            lnE = smpool.tile([P, NST * LC], F32, tag="lnE")
            lns = [nc.scalar.activation(lsum[:], ssum[:], AF.Ln),
                   nc.scalar.activation(lnE[:], endE[:], AF.Ln),
                   nc.scalar.activation(lnS4[:], sum4[:], AF.Ln),
                   nc.scalar.activation(lnB[:], begE[:], AF.Ln)]
            for _li in lns:
                _add_dep_helper(_li.ins, exp_list[-1].ins, True, "ln after exps")

            # ---------------- normalize: x - ln(sum) (DVE) -------------------
            ins_all = wpool.tile([P, NST * LC], BF16, tag="ins_all")
            G_all = wpool.tile([P, NST * LC], F32, tag="g_all")
            E2_all = wpool.tile([P, NST * LC], F32, tag="e2_all")
            for t in range(NST):
                csl = slice(t * LC, (t + 1) * LC)
                ls = lsum[:, t:t + 1]
                nc.vector.tensor_scalar(E2_all[:, csl], lnE[:, csl], ls, -EPS,
                                        AluOpType.subtract, AluOpType.min)
                nc.vector.tensor_scalar(ins_all[:, csl], lnS4[:, csl], ls, None,
                                        AluOpType.subtract)
                nc.vector.tensor_scalar(G_all[:, csl], lnB[:, csl], ls, None,
                                        AluOpType.subtract)

            # ---------------- E2 transpose to rows (PE + DVE copy) -----------
            E2_colT = wpool.tile([LC, S], BF16, tag="e2_colt")
            for t in range(NST):
                csl = slice(t * LC, (t + 1) * LC)
                tp = pst.tile([LC, P], F32, tag="ps_e2t")
                nc.tensor.transpose(tp[:], E2_all[:, csl], eye_sb[:])
                nc.vector.tensor_copy(E2_colT[:, t * P:(t + 1) * P], tp[:])

            # ---------------- A[l,j] = cumsum of inside, via PE --------------
            # A_colT[:, 0] = 0; A_colT[:, 1+j] = A[j]  (C[i]=A[i-1] at col i)
            A_colT = wpool.tile([LC, S + P], F32, tag="a_colt")
            nc.vector.memset(A_colT[:, 0:1], 0.0)
            a_copies = []
            for jc in (1, 0):           # high half first: unblocks tiles 7..4
                jc0 = jc * 512
                ap = psa.tile([P, 512], F32, tag="ps_a")
                tmax = (jc0 + 512) // P
                for ti in range(tmax):
                    o = ti * P - jc0
                    if o < 0:
                        rhs = triw_sb[:, 1024:1536]          # all ones
                    else:
                        rhs = triw_sb[:, 512 - o:1024 - o]   # k <= j' - o
                    nc.tensor.matmul(ap[:LC, :], ins_all[:, ti * LC:(ti + 1) * LC],
                                     rhs, start=ti == 0, stop=ti == tmax - 1)
                cp = nc.vector.tensor_copy(A_colT[:, 1 + jc0:1 + jc0 + 512],
                                           ap[:LC, :])
                a_copies.append(cp)

            # ---------------- C[i] = A[i-1] via PE transpose -----------------
            C_all = wpool.tile([P, NST * LC], F32, tag="c_all")
            ncs_all = wpool.tile([P, NST * LC], F32, tag="ncs_all")
            for t in range(NST - 1, -1, -1):
                csl = slice(t * LC, (t + 1) * LC)
                tp = pst.tile([P, LC], F32, tag="ps_ct")
                nc.tensor.transpose(tp[:], A_colT[:, t * P: (t + 1) * P],
                                    eye_sb[0:LC, 0:LC])
                nc.vector.tensor_copy(C_all[:, csl], tp[:])
                nc.vector.tensor_scalar(ncs_all[:, csl], C_all[:, csl], -1.0,
                                        None, AluOpType.mult)

            # ---------------- broadcasts (DRAM roundtrip) --------------------
            # All destinations are per-partition contiguous (16KB packets).
            A_hi = wpool.tile([P, LC * 512], F32, tag="a_hi")
            E2_hi = wpool.tile([P, LC * 512], BF16, tag="e2_hi")
            A_b = wpool.tile([P, LC * S], F32, tag="a_b")
            E2_b = wpool.tile([P, LC * S], BF16, tag="e2_b")
            A_hi3 = A_hi[:].rearrange("p (l j) -> p l j", l=LC)
            E2_hi3 = E2_hi[:].rearrange("p (l j) -> p l j", l=LC)
            E2_b3 = E2_b[:].rearrange("p (l j) -> p l j", l=LC)

            def _bcast(eng, dst, src_flat, deps, note):
                r = eng.dma_start(out=dst,
                                  in_=src_flat.partition_broadcast(P))
                for dp in deps:
                    _add_dep_helper(r.ins, dp.ins, True, note)
                return r

            # sync: half1 writes + hi-tile broadcast
            aw1 = nc.sync.dma_start(out=a_row_d[:, :], in_=A_colT[:, 513:1025])
            _add_dep_helper(aw1.ins, a_copies[0].ins, True, "a w1")
            aw1b = nc.sync.dma_start(out=a_row2_d[:, 512:1024],
                                     in_=A_colT[:, 513:1025])
            _add_dep_helper(aw1b.ins, a_copies[0].ins, True, "a w1b")
            _bcast(nc.sync, A_hi[:],
                   a_row_d[:, :].rearrange("l j -> (l j)"), [aw1], "a hi")
            # scalar: half0 write + low-label full broadcast
            aw0 = nc.scalar.dma_start(out=a_row2_d[:, 0:512],
                                      in_=A_colT[:, 1:513])
            _add_dep_helper(aw0.ins, a_copies[1].ins, True, "a w0")
            H4 = LC // 2
            _bcast(nc.scalar, A_b[:, :H4 * S],
                   a_row2_d[0:H4, :].rearrange("l j -> (l j)"),
                   [aw0, aw1b], "a full lo-labels")
            _bcast(nc.sync, A_b[:, H4 * S:],
                   a_row2_d[H4:LC, :].rearrange("l j -> (l j)"),
                   [aw0, aw1b], "a full hi-labels")
            # gpsimd: all E2 staging + broadcasts
            ew1 = nc.gpsimd.dma_start(out=e2_row_d[:, :],
                                      in_=E2_colT[:, 512:1024])
            ew1b = nc.gpsimd.dma_start(out=e2_row2_d[:, 512:1024],
                                       in_=E2_colT[:, 512:1024])
            ew0 = nc.gpsimd.dma_start(out=e2_row2_d[:, 0:512],
                                      in_=E2_colT[:, 0:512])
            _bcast(nc.gpsimd, E2_hi[:],
                   e2_row_d[:, :].rearrange("l j -> (l j)"), [ew1], "e2 hi")
            _bcast(nc.gpsimd, E2_b[:, :H4 * S],
                   e2_row2_d[0:H4, :].rearrange("l j -> (l j)"),
                   [ew0, ew1b], "e2 full lo-labels")
            _bcast(nc.gpsimd, E2_b[:, H4 * S:],
                   e2_row2_d[H4:LC, :].rearrange("l j -> (l j)"),
                   [ew0, ew1b], "e2 full hi-labels")

            # ---------------- main span sweep (l-major, bf16) ----------------
            out3 = out[:].rearrange("(t p) f -> t p f", p=P)
            out_eng = [nc.sync, nc.gpsimd]
            # high tiles first (need only the hi broadcasts), then t=0 so its
            # big output DMA overlaps the remaining sweep
            for oi, t in enumerate((7, 6, 5, 4, 0, 1, 2, 3)):
                i0 = t * P
                W = S - i0
                if t >= 4:
                    a_of, a_sb, a_w = i0 - 512, A_hi, 512
                    e2_op = E2_hi3[:, :, i0 - 512:512]
                else:
                    a_of, a_sb, a_w = i0, A_b, S
                    e2_op = E2_b3[:, :, i0:S]
                oc = opool.tile([P, LC * W], OUT_DT, tag="oc")
                u = upool.tile([P, LC * W], OUT_DT, tag="u")
                # labels 6,7 fused on DVE (fp32 1x, keeps ScalarE under its
                # saturation point); 0-5 via ScalarE tsub + DVE 2x min
                for l in (6, 7):
                    ci = t * LC + l
                    nc.vector.tensor_scalar(u[:, l * W:(l + 1) * W],
                                            a_sb[:, l * a_w + a_of:(l + 1) * a_w],
                                            C_all[:, ci:ci + 1],
                                            G_all[:, ci:ci + 1],
                                            AluOpType.subtract, AluOpType.min)
                for l in range(6):
                    ci = t * LC + l
                    tsub = tpool.tile([P, W], OUT_DT, tag="tsub")
                    nc.scalar.activation(tsub[:],
                                         a_sb[:, l * a_w + a_of:(l + 1) * a_w],
                                         AF.Identity, bias=ncs_all[:, ci:ci + 1])
                    nc.vector.tensor_scalar(u[:, l * W:(l + 1) * W], tsub[:],
                                            G_all[:, ci:ci + 1], None,
                                            AluOpType.min)
                u3 = u[:].rearrange("p (l j) -> p l j", j=W)
                oc3 = oc[:].rearrange("p (l j) -> p l j", j=W)
                nc.vector.tensor_tensor(oc3[:], u3[:], e2_op, AluOpType.min)
                dst = out3[t, :, :].rearrange("p (l j) -> p l j", l=LC)[:, :, i0:S]
                out_eng[oi % 2].dma_start(out=dst, in_=oc3)

    nc.compile()
    return nc


def _host_inputs(x, W, b):
    """Build per-core input maps. Core c: batch c//2, label half c%2."""
    x = np.asarray(x, dtype=np.float32)
    W = np.asarray(W, dtype=np.float32)
    b = np.asarray(b, dtype=np.float32)

    Wb = np.concatenate([W, b[None, :]], axis=0)          # (401, 65)
    eye = np.eye(P, dtype=np.float32)
    triw = np.zeros((P, 1536), np.float32)
    cc = np.arange(1536)[None, :]
    kk = np.arange(P)[:, None]
    triw[kk <= cc - 512] = 1.0
    triw = triw.astype(ml_dtypes.bfloat16)

    in_maps = []
    for c in range(8):
        bb, h = c // 2, c % 2
        cols = []
        for l in range(LC):
            base = 1 + 4 * (h * LC + l)
            cols.extend(range(base, base + 4))
        xTb = np.concatenate([x[bb].T, np.ones((1, S), np.float32)], axis=0)
        wcat = np.concatenate([Wb, Wb[:, cols]], axis=1)          # (401, 97)
        xp = np.zeros((NKT * P, S), np.float32)
        xp[:H + 1] = xTb
        # [P, (q, ki, 256)]: s-quarter major, so quarter DMAs are contiguous
        xp = (xp.reshape(NKT, P, 4, QC).transpose(1, 2, 0, 3)
                .reshape(P, NKT * S))
        wp = np.zeros((NKT * P, NW), np.float32)
        wp[:H + 1] = wcat
        wp = np.ascontiguousarray(
            wp.reshape(NKT, P, NW).transpose(1, 0, 2).reshape(P, NKT * NW))
        wp = np.concatenate([wp.astype(ml_dtypes.bfloat16), triw], axis=1)
        in_maps.append({
            "xTb": np.ascontiguousarray(xp).astype(ml_dtypes.bfloat16),
            "Wcat": wp,
            "eye": eye,
        })
    return in_maps


def _from_out_dt(a):
    if OUT_DT == F32:
        return a
    return (a.astype(np.uint32) << 16).view(np.float32)


def kernel(x, mask, W, b, _collect=None):
    global _CACHED_NC
    if _CACHED_NC is None:
        _CACHED_NC = _build()
    nc = _CACHED_NC
    in_maps = _host_inputs(x, W, b)
    res = run_bass_kernel_spmd(nc, in_maps, list(range(8)))
    if _collect is not None:
        _collect.append(res)
    outf = np.empty((B, S, S, NL), dtype=np.float32)
    for c in range(8):
        bb, h = c // 2, c % 2
        o = res.results[c]["out"]
        if o.dtype != np.float32:
            o = _from_out_dt(o.view(OUT_NP) if o.dtype != OUT_NP else o)
        o = o.reshape(S, LC, S)                       # [i, l, j]
        outf[bb, :, :, h * LC:(h + 1) * LC] = o.transpose(0, 2, 1)
    # full constant lower triangle filled on host (device only computes
    # j >= i0 per row tile and leaves j < i garbage within the tile)
    for i in range(1, S):
        outf[:, i, :i, :] = NEG
    return outf
